# revision 17
# baseline (speedup 1.0000x reference)
"""Trainium2 Bass kernel for nn_MultiHeadedAttention (B=8,S=1024,E=1024,H=16).

Strategy: batch-parallel across 8 NeuronCores (1 batch element per core).

Per core (batch b), with host-side pre-transposition of inputs/weights:
  QT[e,s] = sum_f WqT[f,e] * XqT[f,s]   (fp32r matmuls, K accumulated in PSUM)
  KT, V similarly (V stored in a per-head-sliced layout).
  Pass A (attn output): per head, scores[i,j] = QT_h.T @ KT_h, exp(x/8) with
    fused row-sum accum -> recip -> attn = exp*recip + D  -> DMA out.
  T2 = (D @ V)^T[e,i] via lhsT=V-slices, rhs=DT.
  Pass B (context): scoresT[j,i] = KT_h.T @ QT_h, exp, then
    XVu^T[d,i] = v_h.T @ expT and denom = ones.T @ expT accumulated over j;
    normalize columns via reciprocal + partition_broadcast; accumulate into
    XVT[e,s] (pre-loaded with T2).
  Phase C: out[i,eo] = sum_e XVT[e,i] * WoT[e,eo]; bo added on host.
"""

import math
import ml_dtypes
import numpy as np

B, S, E, H = 8, 1024, 1024, 16
HD = E // H
NCORES = 8
NT = 8  # number of 128-tiles along S or E

_CACHE = {}


def _numpy_fallback(query, key, value, mask, gnn_adj, dist_score,
                    Wq, bq, Wk, bk, Wv, bv, Wo, bo):
    """Pure-numpy reference path (used only for unexpected inputs, e.g. a
    mask that is not all ones)."""
    def split_heads(x):
        b, s, _ = x.shape
        return x.reshape(b, s, H, HD).transpose(0, 2, 1, 3)

    q = split_heads(query @ Wq.T + bq)
    k = split_heads(key @ Wk.T + bk)
    v = split_heads(value @ Wv.T + bv)
    scores = np.einsum("bhqd,bhkd->bhqk", q, k) / math.sqrt(HD)
    m = mask[:, None]
    scores = np.where(m == 0, -np.inf, scores * m)
    scores = scores - scores.max(-1, keepdims=True)
    e = np.exp(scores)
    attn = e / e.sum(-1, keepdims=True)
    attn = attn + dist_score[:, None] + gnn_adj[:, None]
    xv = np.einsum("bhqk,bhkd->bhqd", attn, v)
    xv = xv.transpose(0, 2, 1, 3).reshape(query.shape[0], -1, E)
    out = xv @ Wo.T + bo
    return out.astype(np.float32), attn.astype(np.float32)


def _build():
    import concourse.bacc as bacc
    import concourse.mybir as mybir
    import concourse.tile as tile

    F32R = mybir.dt.float32r
    F32 = mybir.dt.float32
    BF16 = mybir.dt.bfloat16
    AF = mybir.ActivationFunctionType
    OP = mybir.AluOpType

    nc = bacc.Bacc(trn_type="TRN2")

    # ---- DRAM I/O (per core) ----
    xqt = nc.dram_tensor("xqt", [E, S], F32R, kind="ExternalInput")
    xkt = nc.dram_tensor("xkt", [E, S], F32R, kind="ExternalInput")
    xvt = nc.dram_tensor("xvt", [E, S], F32R, kind="ExternalInput")
    wqt = nc.dram_tensor("wqt", [E, E], F32R, kind="ExternalInput")
    wkt = nc.dram_tensor("wkt", [E, E], F32R, kind="ExternalInput")
    wvt = nc.dram_tensor("wvt", [E, E], F32R, kind="ExternalInput")
    wot = nc.dram_tensor("wot", [E, E], F32R, kind="ExternalInput")
    bq_d = nc.dram_tensor("bqr", [NT, 128, 1], F32, kind="ExternalInput")
    bk_d = nc.dram_tensor("bkr", [NT, 128, 1], F32, kind="ExternalInput")
    bv_d = nc.dram_tensor("bvr", [1, E], F32, kind="ExternalInput")
    d_d = nc.dram_tensor("dmat", [S, S], F32R, kind="ExternalInput")
    dt_d = nc.dram_tensor("dmatt", [S, S], BF16, kind="ExternalInput")
    ones_d = nc.dram_tensor("ones", [128, 1], BF16, kind="ExternalInput")
    out_d = nc.dram_tensor("out", [S, E], F32, kind="ExternalOutput")
    attn_d = nc.dram_tensor("attn", [H, S, S], F32, kind="ExternalOutput")

    with tile.TileContext(nc) as tc:
        from contextlib import ExitStack
        with ExitStack() as ctx:
            # ---- persistent pools ----
            per = ctx.enter_context(tc.tile_pool(name="per", bufs=1))
            qt = per.tile([128, NT, S], F32R, tag="qt")
            kt = per.tile([128, NT, S], F32R, tag="kt")
            vp = per.tile([128, NT, H, HD], BF16, tag="vp")
            ones_t = per.tile([128, 1], BF16, tag="ones")
            xvt_sb = per.tile([128, NT, S], F32R, tag="xvt")

            bqt = per.tile([128, NT], F32, tag="bqt")
            bkt = per.tile([128, NT], F32, tag="bkt")
            bvrow = per.tile([1, E], F32, tag="bvrow")
            bvb = per.tile([128, E], F32, tag="bvb")

            for t in range(NT):
                nc.sync.dma_start(bqt[:, t:t + 1], bq_d[t])
                nc.sync.dma_start(bkt[:, t:t + 1], bk_d[t])
            nc.sync.dma_start(bvrow[:], bv_d[:])
            nc.gpsimd.partition_broadcast(bvb[:], bvrow[:])
            nc.sync.dma_start(ones_t[:], ones_d[:])

            # ================= Phase 0: projections =================
            with (
                tc.tile_pool(name="win", bufs=8) as win,
                tc.tile_pool(name="xin", bufs=8) as xin,
                tc.tile_pool(name="ps0", bufs=4, space="PSUM") as ps0,
            ):
                # --- Q and K projections -> qt/kt [e_p, e_t, s] ---
                for (w_dram, x_dram, dst, bias) in (
                    (wqt, xqt, qt, bqt), (wkt, xkt, kt, bkt),
                ):
                    w_tiles, x_tiles = [], []
                    for f in range(NT):
                        wtl = win.tile([128, E], F32R, tag="w")
                        nc.sync.dma_start(wtl[:], w_dram[f * 128:(f + 1) * 128, :])
                        w_tiles.append(wtl)
                        xtl = xin.tile([128, S], F32R, tag="x")
                        nc.sync.dma_start(xtl[:], x_dram[f * 128:(f + 1) * 128, :])
                        x_tiles.append(xtl)
                    for et in range(NT):
                        for sh in range(2):
                            ps = ps0.tile([128, 512], F32, tag="p0")
                            for f in range(NT):
                                nc.tensor.matmul(
                                    ps[:], w_tiles[f][:, et * 128:(et + 1) * 128],
                                    x_tiles[f][:, sh * 512:(sh + 1) * 512],
                                    start=(f == 0), stop=(f == NT - 1))
                            nc.vector.tensor_scalar_add(
                                dst[:, et, sh * 512:(sh + 1) * 512], ps[:],
                                bias[:, et:et + 1])

                # --- V projection -> vp [s_p, s_t, h, d(+1)] ---
                w_tiles, x_tiles = [], []
                for f in range(NT):
                    wtl = win.tile([128, E], F32R, tag="w")
                    nc.sync.dma_start(wtl[:], wvt[f * 128:(f + 1) * 128, :])
                    w_tiles.append(wtl)
                    xtl = xin.tile([128, S], F32R, tag="x")
                    nc.sync.dma_start(xtl[:], xvt[f * 128:(f + 1) * 128, :])
                    x_tiles.append(xtl)
                for st in range(NT):
                    for eh in range(2):
                        ps = ps0.tile([128, 512], F32, tag="p0")
                        for f in range(NT):
                            nc.tensor.matmul(
                                ps[:], x_tiles[f][:, st * 128:(st + 1) * 128],
                                w_tiles[f][:, eh * 512:(eh + 1) * 512],
                                start=(f == 0), stop=(f == NT - 1))
                        for hh in range(8):
                            h = eh * 8 + hh
                            nc.vector.tensor_add(
                                vp[:, st, h, 0:HD], ps[:, hh * HD:(hh + 1) * HD],
                                bvb[:, h * HD:(h + 1) * HD])

            # ================= T2 = (D @ V)^T into xvt_sb =================
            with (
                tc.tile_pool(name="dtin", bufs=1) as dtin,
                tc.tile_pool(name="pst2", bufs=2, space="PSUM") as pst2,
            ):
                dt_tiles = []
                for j in range(NT):
                    dtl = dtin.tile([128, S], BF16, tag=f"dt{j}")
                    nc.sync.dma_start(dtl[:], dt_d[j * 128:(j + 1) * 128, :])
                    dt_tiles.append(dtl)
                for et in range(NT):
                    for ih in range(2):
                        ps = pst2.tile([128, 512], F32, tag="t2")
                        for j in range(NT):
                            rhs = dt_tiles[j][:, ih * 512:(ih + 1) * 512]
                            nc.tensor.matmul(
                                ps[0:64, :], vp[:, j, 2 * et], rhs,
                                start=(j == 0), stop=(j == NT - 1))
                            nc.tensor.matmul(
                                ps[64:128, :], vp[:, j, 2 * et + 1], rhs,
                                tile_position=(0, 64),
                                start=(j == 0), stop=(j == NT - 1))
                        nc.vector.tensor_copy(
                            xvt_sb[:, et, ih * 512:(ih + 1) * 512], ps[:])

            # ============== Interleaved pass A (attn out) + pass B (xv) ====
            with (
                tc.tile_pool(name="din", bufs=2) as din,
                tc.tile_pool(name="ea", bufs=2) as ea,
                tc.tile_pool(name="aa", bufs=2) as aa,
                tc.tile_pool(name="eb", bufs=2) as eb,
                tc.tile_pool(name="nrm", bufs=2) as nrm,
                tc.tile_pool(name="psa", bufs=1, space="PSUM") as psa,
                tc.tile_pool(name="psb", bufs=1, space="PSUM") as psb,
                tc.tile_pool(name="psxv", bufs=2, space="PSUM") as psxv,
            ):
                def head_slices(h):
                    po = (h % 2) * 64
                    return (qt[po:po + 64, h // 2, :], kt[po:po + 64, h // 2, :])

                def pass_a_slice(i):
                    """Attn-output work for row-tile i, all heads."""
                    dtl = din.tile([128, S], F32R, tag="d")
                    nc.sync.dma_start(dtl[:], d_d[i * 128:(i + 1) * 128, :])
                    for p in range(8):
                        for h in (2 * p, 2 * p + 1):
                            qh, kh = head_slices(h)
                            ps = psa.tile([128, S], F32, tag="sa")
                            for jh in range(2):
                                nc.tensor.matmul(
                                    ps[:, jh * 512:(jh + 1) * 512],
                                    qh[:, i * 128:(i + 1) * 128],
                                    kh[:, jh * 512:(jh + 1) * 512],
                                    start=True, stop=True)
                            e_sb = ea.tile([128, S], F32R, tag="e")
                            den = nrm.tile([128, 1], F32, tag="den")
                            nc.scalar.activation(e_sb[:], ps[:], AF.Exp,
                                                 scale=0.125, accum_out=den[:])
                            rec = nrm.tile([128, 1], F32, tag="rec")
                            nc.vector.reciprocal(rec[:], den[:])
                            a_sb = aa.tile([128, S], F32, tag="a")
                            nc.vector.scalar_tensor_tensor(
                                a_sb[:], e_sb[:], rec[:], dtl[:],
                                op0=OP.mult, op1=OP.add)
                            nc.sync.dma_start(
                                attn_d[h, i * 128:(i + 1) * 128, :], a_sb[:])

                def pass_b_pair(p):
                    """Context (xv) work for head pair p.

                    Data matmuls are column-packed: h0 -> psum partitions
                    0:64, h1 -> 64:128 (tile_position=(0,64)), matching the
                    xvt_sb head layout so no DVE partition shifts are needed.
                    Denominators come from separate M=1 matmuls against the
                    vp ones column (h0 -> partition 0, h1 -> partition 32).
                    """
                    h0, h1 = 2 * p, 2 * p + 1
                    xv_ps, den_ps = {}, {}
                    for ih in range(2):
                        xv_ps[ih] = psxv.tile([128, 512], F32, tag="xv",
                                              name=f"xv_{p}_{ih}")
                        den_ps[ih] = psxv.tile([33, 512], F32, tag="den",
                                               name=f"den_{p}_{ih}")
                    for j in range(NT):
                        for h, poff, doff in ((h0, 0, 0), (h1, 64, 32)):
                            qh, kh = head_slices(h)
                            ps = psb.tile([128, S], F32, tag="sb")
                            for ih in range(2):
                                nc.tensor.matmul(
                                    ps[:, ih * 512:(ih + 1) * 512],
                                    kh[:, j * 128:(j + 1) * 128],
                                    qh[:, ih * 512:(ih + 1) * 512],
                                    start=True, stop=True)
                            et_sb = eb.tile([128, S], BF16, tag="et")
                            nc.scalar.activation(et_sb[:], ps[:], AF.Exp,
                                                 scale=0.125)
                            for ih in range(2):
                                ets = et_sb[:, ih * 512:(ih + 1) * 512]
                                nc.tensor.matmul(
                                    xv_ps[ih][poff:poff + 64, :],
                                    vp[:, j, h], ets,
                                    tile_position=(
                                        (0, 64) if poff else None),
                                    start=(j == 0), stop=(j == NT - 1))
                                nc.tensor.matmul(
                                    den_ps[ih][doff:doff + 1, :],
                                    ones_t[:], ets,
                                    tile_position=(
                                        (0, 32) if doff else None),
                                    start=(j == 0), stop=(j == NT - 1))
                    for ih in range(2):
                        tnorm = nrm.tile([128, 512], F32R, tag="tn")
                        for h, poff, doff in ((h0, 0, 0), (h1, 64, 32)):
                            rrow = nrm.tile([1, 512], F32, tag="rrow",
                                            name=f"rrow_{p}_{ih}_{h}")
                            nc.vector.reciprocal(
                                rrow[:], den_ps[ih][doff:doff + 1, :])
                            bc = nrm.tile([128, 512], F32, tag="bc",
                                          name=f"bc_{p}_{ih}_{h}")
                            nc.gpsimd.partition_broadcast(bc[:], rrow[:])
                            nc.vector.tensor_mul(
                                tnorm[poff:poff + 64, :],
                                xv_ps[ih][poff:poff + 64, :],
                                bc[poff:poff + 64, :])
                        dst = xvt_sb[:, p, ih * 512:(ih + 1) * 512]
                        nc.vector.tensor_add(dst, dst, tnorm[:])

                for i in range(NT):
                    pass_a_slice(i)
                    pass_b_pair(i)

            # ================= Phase C: output projection =================
            with (
                tc.tile_pool(name="woin", bufs=8) as woin,
                tc.tile_pool(name="oo", bufs=3) as oo,
                tc.tile_pool(name="psc", bufs=4, space="PSUM") as psc,
            ):
                wo_tiles = []
                for f in range(NT):
                    wtl = woin.tile([128, E], F32R, tag="wo")
                    nc.sync.dma_start(wtl[:], wot[f * 128:(f + 1) * 128, :])
                    wo_tiles.append(wtl)
                for it in range(NT):
                    for oh in range(2):
                        ps = psc.tile([128, 512], F32, tag="pc")
                        for f in range(NT):
                            nc.tensor.matmul(
                                ps[:], xvt_sb[:, f, it * 128:(it + 1) * 128],
                                wo_tiles[f][:, oh * 512:(oh + 1) * 512],
                                start=(f == 0), stop=(f == NT - 1))
                        o_sb = oo.tile([128, 512], F32, tag="o")
                        nc.vector.tensor_copy(o_sb[:], ps[:])
                        nc.sync.dma_start(
                            out_d[it * 128:(it + 1) * 128,
                                  oh * 512:(oh + 1) * 512], o_sb[:])

    nc.finalize()
    return nc


def kernel(**inputs):
    query = np.asarray(inputs["query"], np.float32)
    key = np.asarray(inputs["key"], np.float32)
    value = np.asarray(inputs["value"], np.float32)
    mask = np.asarray(inputs["mask"], np.float32)
    gnn_adj = np.asarray(inputs["gnn_adj"], np.float32)
    dist_score = np.asarray(inputs["dist_score"], np.float32)
    Wq = np.asarray(inputs["Wq"], np.float32)
    bq = np.asarray(inputs["bq"], np.float32)
    Wk = np.asarray(inputs["Wk"], np.float32)
    bk = np.asarray(inputs["bk"], np.float32)
    Wv = np.asarray(inputs["Wv"], np.float32)
    bv = np.asarray(inputs["bv"], np.float32)
    Wo = np.asarray(inputs["Wo"], np.float32)
    bo = np.asarray(inputs["bo"], np.float32)

    if (query.shape != (B, S, E) or mask.shape != (B, S, S)
            or not np.all(mask == 1.0)):
        return _numpy_fallback(query, key, value, mask, gnn_adj, dist_score,
                               Wq, bq, Wk, bk, Wv, bv, Wo, bo)

    from concourse.bass_utils import run_bass_kernel_spmd

    if "nc" not in _CACHE:
        _CACHE["nc"] = _build()
    nc = _CACHE["nc"]

    wqt = np.ascontiguousarray(Wq.T)
    wkt = np.ascontiguousarray(Wk.T)
    wvt = np.ascontiguousarray(Wv.T)
    wot = np.ascontiguousarray(Wo.T)
    bqr = np.ascontiguousarray(bq.reshape(NT, 128, 1))
    bkr = np.ascontiguousarray(bk.reshape(NT, 128, 1))
    bvr = np.ascontiguousarray(bv.reshape(1, E))
    D = dist_score + gnn_adj

    in_maps = []
    for c in range(NCORES):
        in_maps.append({
            "xqt": np.ascontiguousarray(query[c].T),
            "xkt": np.ascontiguousarray(key[c].T),
            "xvt": np.ascontiguousarray(value[c].T),
            "wqt": wqt, "wkt": wkt, "wvt": wvt, "wot": wot,
            "bqr": bqr, "bkr": bkr, "bvr": bvr,
            "dmat": np.ascontiguousarray(D[c]),
            "dmatt": np.ascontiguousarray(D[c].T).astype(ml_dtypes.bfloat16),
            "ones": np.ones((128, 1), ml_dtypes.bfloat16),
        })

    import os
    trace = bool(int(os.environ.get("KERNEL_PROFILE", "0")))
    res = run_bass_kernel_spmd(nc, in_maps, core_ids=list(range(NCORES)),
                               trace=trace)
    _CACHE["last_results"] = res

    out = np.stack([res.results[c]["out"] for c in range(NCORES)])
    attn = np.stack([res.results[c]["attn"] for c in range(NCORES)])
    out = out + bo[None, None, :]
    return out.astype(np.float32), attn.astype(np.float32)
